# revision 2
# baseline (speedup 1.0000x reference)
"""AdaptiveSlotPruning Trainium2 kernel (8 NeuronCores, pure data parallel).

Full-input interface: kernel(**inputs) takes the unsharded numpy inputs
(slots [128,12,256], masks [128,16384,12], W1 [1,16], b1 [16], W2 [16,1],
b2 [1]) and returns (pruned_slots, pruned_masks, gates, utilization) as
full-shape float32 arrays, matching reference().

Internally the batch dim (128) is sharded 16-per-core across 8 cores; the
gate-MLP params are replicated. No cross-core communication.
"""

import sys

sys.path.insert(0, "/opt/trn_rl_repo")

import numpy as np

import concourse.bass as bass  # noqa: F401  (re-exported for tests)
import concourse.tile as tile
from concourse import bacc, mybir
from concourse.bass_utils import run_bass_kernel_spmd

F32 = mybir.dt.float32
BF16 = mybir.dt.bfloat16

# Problem constants (hardcoded per spec).
B = 128          # full batch
NC = 8           # cores
BL = B // NC     # 16 batches per core
N = 16384
K = 12
D = 256
P = 128          # partitions
NI = N // P      # 128 inner rows per partition
FREE = NI * K    # 1536 free elements per partition per batch
GB = 4           # batches per group
NG = BL // GB    # 4 groups
CHUNK = 512      # matmul moving free-dim max (fp32)


def build_nc(compute_dtype: str = "f32"):
    """Build the per-core Bass program. Same program on all 8 cores."""
    bf16 = compute_dtype == "bf16"
    cdt = BF16 if bf16 else F32

    nc = bacc.Bacc("TRN2", target_bir_lowering=False, debug=False, num_devices=NC)

    masks = nc.dram_tensor("masks", [BL, N, K], F32, kind="ExternalInput").ap()
    slots = nc.dram_tensor("slots", [BL, K, D], F32, kind="ExternalInput").ap()
    w1n = nc.dram_tensor("w1n", [16, 16], F32, kind="ExternalInput").ap()
    b1r = nc.dram_tensor("b1r", [16, 16], F32, kind="ExternalInput").ap()
    w2r = nc.dram_tensor("w2r", [16, 16], F32, kind="ExternalInput").ap()
    b2r = nc.dram_tensor("b2r", [16, 1], F32, kind="ExternalInput").ap()
    sel4 = nc.dram_tensor("sel4", [P, GB, GB], cdt, kind="ExternalInput").ap()
    brow4 = nc.dram_tensor("brow4", [GB, GB, P], F32, kind="ExternalInput").ap()
    eye16 = nc.dram_tensor("eye16", [16, 16], F32, kind="ExternalInput").ap()

    p_slots = nc.dram_tensor("pruned_slots", [BL, K, D], F32, kind="ExternalOutput").ap()
    p_masks = nc.dram_tensor("pruned_masks", [BL, N, K], F32, kind="ExternalOutput").ap()
    gates_o = nc.dram_tensor("gates", [BL, K], F32, kind="ExternalOutput").ap()
    util_o = nc.dram_tensor("util", [BL, K], F32, kind="ExternalOutput").ap()

    with tile.TileContext(nc) as tc:
        from contextlib import ExitStack

        with ExitStack() as ctx:
            consts = ctx.enter_context(tc.tile_pool(name="consts", bufs=1))
            mpool = ctx.enter_context(tc.tile_pool(name="mpool", bufs=10))
            srpool = ctx.enter_context(tc.tile_pool(name="srpool", bufs=6))
            gpool = ctx.enter_context(tc.tile_pool(name="gpool", bufs=8))
            mlp = ctx.enter_context(tc.tile_pool(name="mlp", bufs=2))
            slotp = ctx.enter_context(tc.tile_pool(name="slotp", bufs=4))
            psu = ctx.enter_context(tc.tile_pool(name="psu", bufs=1, space="PSUM"))
            pss = ctx.enter_context(tc.tile_pool(name="pss", bufs=2, space="PSUM"))

            # --- constants to SBUF (once) ---
            sb_w1n = consts.tile([16, 16], F32)
            nc.sync.dma_start(sb_w1n[:], w1n)
            sb_b1r = consts.tile([16, 16], F32)
            nc.sync.dma_start(sb_b1r[:], b1r)
            sb_w2r = consts.tile([16, 16], F32)
            nc.sync.dma_start(sb_w2r[:], w2r)
            sb_b2r = consts.tile([16, 1], F32)
            nc.sync.dma_start(sb_b2r[:], b2r)
            sb_sel4 = consts.tile([P, GB, GB], cdt)
            nc.sync.dma_start(sb_sel4[:], sel4)
            sb_brow4 = consts.tile([GB, GB, P], F32)
            nc.sync.dma_start(sb_brow4[:], brow4)
            sb_eye = consts.tile([16, 16], F32)
            nc.sync.dma_start(sb_eye[:], eye16)
            # gates transposed to k-partitions, for slot scaling: [K, BL]
            sb_gt = consts.tile([K, BL], F32)

            for g in range(NG):
                pu = psu.tile([GB, FREE], F32, tag="pu")
                m_tiles = []
                for i in range(GB):
                    b = g * GB + i
                    mt = mpool.tile([P, FREE], cdt, tag="m")
                    src = masks[b].rearrange("(p ni) k -> p (ni k)", p=P)
                    if bf16:
                        nc.gpsimd.dma_start(mt[:], src)  # SWDGE casts f32->bf16
                    else:
                        nc.sync.dma_start(mt[:], src)
                    m_tiles.append(mt)
                    for j in range(FREE // CHUNK):
                        nc.tensor.matmul(
                            pu[:, j * CHUNK:(j + 1) * CHUNK],
                            lhsT=sb_sel4[:, i, :],
                            rhs=mt[:, j * CHUNK:(j + 1) * CHUNK],
                            start=(i == 0),
                            stop=(i == GB - 1),
                        )

                # --- gate MLP for this group's 4 batches (partitions 0..3) ---
                ug = mlp.tile([GB, K], F32, tag="ug")  # raw sums over n
                nc.vector.reduce_sum(
                    ug[:],
                    pu[:].rearrange("p (ni k) -> p k ni", k=K),
                    axis=mybir.AxisListType.X,
                )
                # h = relu(u_raw * (W1/N) + b1): [GB, 16, K]
                h = mlp.tile([GB, 16, K], F32, tag="h")
                nc.vector.tensor_mul(
                    h[:],
                    ug[:][:, None, :].broadcast_to([GB, 16, K]),
                    sb_w1n[:GB, :][:, :, None].broadcast_to([GB, 16, K]),
                )
                nc.vector.tensor_add(
                    h[:], h[:], sb_b1r[:GB, :][:, :, None].broadcast_to([GB, 16, K])
                )
                h2 = mlp.tile([GB, 16, K], F32, tag="h2")
                nc.scalar.activation(h2[:], h[:], mybir.ActivationFunctionType.Relu)
                # hw = h2 * W2 ; gates_pre = sum_j hw
                hw = mlp.tile([GB, 16, K], F32, tag="hw")
                nc.vector.tensor_mul(
                    hw[:], h2[:], sb_w2r[:GB, :][:, :, None].broadcast_to([GB, 16, K])
                )
                gpre = mlp.tile([GB, K], F32, tag="gpre")
                nc.vector.reduce_sum(
                    gpre[:],
                    hw[:].rearrange("p j k -> p k j"),
                    axis=mybir.AxisListType.X,
                )
                ggrp = mlp.tile([GB, K], F32, tag="ggrp")
                nc.scalar.activation(
                    ggrp[:],
                    gpre[:],
                    mybir.ActivationFunctionType.Sigmoid,
                    bias=sb_b2r[:GB, :],
                    scale=1.0,
                )
                # outputs: gates + utilization rows for this group
                nc.scalar.dma_start(gates_o[g * GB:(g + 1) * GB, :], ggrp[:])
                utilg = mlp.tile([GB, K], F32, tag="utilg")
                nc.scalar.mul(utilg[:], ug[:], 1.0 / N)
                nc.scalar.dma_start(util_o[g * GB:(g + 1) * GB, :], utilg[:])

                # gates -> [K, GB] slice of sb_gt (for slot scaling)
                pgt = pss.tile([K, GB], F32, tag="pgt")
                nc.tensor.transpose(pgt[:], ggrp[:], sb_eye[:GB, :GB])
                nc.scalar.copy(sb_gt[:, g * GB:(g + 1) * GB], pgt[:])

                # broadcast each batch's gates to all 128 partitions
                greps = []
                for i in range(GB):
                    pg = pss.tile([P, K], F32, tag="pg")
                    nc.tensor.matmul(
                        pg[:], lhsT=sb_brow4[:, i, :], rhs=ggrp[:],
                        start=True, stop=True,
                    )
                    grep = gpool.tile([P, K], cdt, tag="grep")
                    nc.scalar.copy(grep[:], pg[:])
                    greps.append(grep)

                # --- normalize + scale this group's mask tiles ---
                for i in range(GB):
                    b = g * GB + i
                    mt = m_tiles[i]
                    m3 = mt[:].rearrange("p (ni k) -> p ni k", k=K)
                    nc.vector.tensor_mul(
                        m3, m3, greps[i][:][:, None, :].broadcast_to([P, NI, K])
                    )
                    s = srpool.tile([P, NI], F32, tag="s")
                    nc.vector.reduce_sum(s[:], m3, axis=mybir.AxisListType.X)
                    r = srpool.tile([P, NI], F32, tag="r")
                    nc.vector.reciprocal_approx_fast(out=r[:], in_=s[:])
                    if bf16:
                        rb = srpool.tile([P, NI], BF16, tag="rb")
                        nc.scalar.copy(rb[:], r[:])
                        rr = rb
                    else:
                        rr = r
                    nc.vector.tensor_mul(
                        m3, m3, rr[:][:, :, None].broadcast_to([P, NI, K])
                    )
                    dst = p_masks[b].rearrange("(p ni) k -> p (ni k)", p=P)
                    if bf16:
                        nc.gpsimd.dma_start(dst, mt[:])  # SWDGE casts bf16->f32
                    else:
                        nc.scalar.dma_start(dst, mt[:])

            # --- slots: pruned_slots[b] = slots[b] * gates[b,k] ---
            for b in range(BL):
                st = slotp.tile([K, D], F32, tag="st")
                nc.sync.dma_start(st[:], slots[b])
                nc.vector.tensor_scalar(
                    out=st[:], in0=st[:], scalar1=sb_gt[:, b:b + 1], scalar2=None,
                    op0=mybir.AluOpType.mult,
                )
                nc.scalar.dma_start(p_slots[b], st[:])

    return nc


def make_host_consts(W1, b1, W2, b2, compute_dtype: str = "f32"):
    cnp = np.dtype("bfloat16") if compute_dtype == "bf16" else np.float32
    try:
        import ml_dtypes  # noqa: F401
        if compute_dtype == "bf16":
            cnp = ml_dtypes.bfloat16
    except ImportError:
        pass
    w1n = np.tile((np.asarray(W1, np.float32).reshape(-1) / N), (16, 1)).astype(np.float32)
    b1r = np.tile(np.asarray(b1, np.float32).reshape(-1), (16, 1)).astype(np.float32)
    w2r = np.tile(np.asarray(W2, np.float32).reshape(-1), (16, 1)).astype(np.float32)
    b2r = np.full((16, 1), float(np.asarray(b2).reshape(-1)[0]), np.float32)
    sel4 = np.zeros((P, GB, GB), np.float32)
    for i in range(GB):
        sel4[:, i, i] = 1.0
    brow4 = np.zeros((GB, GB, P), np.float32)
    for i in range(GB):
        brow4[i, i, :] = 1.0
    eye = np.eye(16, dtype=np.float32)
    return {
        "w1n": w1n,
        "b1r": b1r,
        "w2r": w2r,
        "b2r": b2r,
        "sel4": sel4.astype(cnp),
        "brow4": brow4,
        "eye16": eye,
    }


_CACHE = {}


def _get_nc(compute_dtype: str):
    key = compute_dtype
    if key not in _CACHE:
        nc = build_nc(compute_dtype)
        nc.compile()
        _CACHE[key] = nc
    return _CACHE[key]


COMPUTE_DTYPE = "f32"


def kernel(slots, masks, W1, b1, W2, b2, _trace=False, _trace_kwargs=None):
    nc = _get_nc(COMPUTE_DTYPE)
    consts = make_host_consts(W1, b1, W2, b2, COMPUTE_DTYPE)
    masks = np.ascontiguousarray(np.asarray(masks, np.float32))
    slots = np.ascontiguousarray(np.asarray(slots, np.float32))
    in_maps = []
    for c in range(NC):
        m = dict(consts)
        m["masks"] = masks[c * BL:(c + 1) * BL]
        m["slots"] = slots[c * BL:(c + 1) * BL]
        in_maps.append(m)
    kw = {}
    if _trace:
        kw["trace"] = True
        kw.update(_trace_kwargs or {})
    res = run_bass_kernel_spmd(nc, in_maps, list(range(NC)), **kw)
    outs = res.results
    pruned_slots = np.concatenate([outs[c]["pruned_slots"] for c in range(NC)], axis=0)
    pruned_masks = np.concatenate([outs[c]["pruned_masks"] for c in range(NC)], axis=0)
    gates = np.concatenate([outs[c]["gates"] for c in range(NC)], axis=0)
    util = np.concatenate([outs[c]["util"] for c in range(NC)], axis=0)
    kernel.last_results = res
    return (
        pruned_slots.astype(np.float32),
        pruned_masks.astype(np.float32),
        gates.astype(np.float32),
        util.astype(np.float32),
    )


# revision 6
# speedup vs baseline: 1.1527x; 1.1527x over previous
"""AdaptiveSlotPruning Trainium2 kernel (8 NeuronCores, pure data parallel).

Full-input interface: kernel(**inputs) takes the unsharded numpy inputs
(slots [128,12,256], masks [128,16384,12], W1 [1,16], b1 [16], W2 [16,1],
b2 [1]) and returns (pruned_slots, pruned_masks, gates, utilization) as
full-shape float32 arrays, matching reference().

Internally the batch dim (128) is sharded 16-per-core across 8 cores; the
gate-MLP params are replicated. No cross-core communication.
"""

import sys

sys.path.insert(0, "/opt/trn_rl_repo")

import numpy as np

import concourse.bass as bass  # noqa: F401  (re-exported for tests)
import concourse.tile as tile
from concourse import bacc, mybir
from concourse.bass_utils import run_bass_kernel_spmd

F32 = mybir.dt.float32
BF16 = mybir.dt.bfloat16

# Problem constants (hardcoded per spec).
B = 128          # full batch
NC = 8           # cores
BL = B // NC     # 16 batches per core
N = 16384
K = 12
D = 256
P = 128          # partitions
NI = N // P      # 128 inner rows per partition
FREE = NI * K    # 1536 free elements per partition per batch
GB = 4           # max batches per group
GROUPS = [2, 2, 4, 4, 4]   # group sizes (sum = BL); small first groups cut pipeline fill
CHUNK = 512      # matmul moving free-dim max (fp32)


def build_nc(compute_dtype: str = "f32"):
    """Build the per-core Bass program. Same program on all 8 cores."""
    bf16 = compute_dtype == "bf16"
    cdt = BF16 if bf16 else F32

    nc = bacc.Bacc("TRN2", target_bir_lowering=False, debug=False, num_devices=NC)

    masks = nc.dram_tensor("masks", [BL, N, K], F32, kind="ExternalInput").ap()
    slots = nc.dram_tensor("slots", [BL, K, D], F32, kind="ExternalInput").ap()
    w1n = nc.dram_tensor("w1n", [16, 16], F32, kind="ExternalInput").ap()
    b1r = nc.dram_tensor("b1r", [16, 16], F32, kind="ExternalInput").ap()
    w2r = nc.dram_tensor("w2r", [16, 16], F32, kind="ExternalInput").ap()
    b2r = nc.dram_tensor("b2r", [16, 1], F32, kind="ExternalInput").ap()
    sel4 = nc.dram_tensor("sel4", [P, GB, GB], cdt, kind="ExternalInput").ap()
    brow4 = nc.dram_tensor("brow4", [GB, GB, P], F32, kind="ExternalInput").ap()
    eye16 = nc.dram_tensor("eye16", [16, 16], F32, kind="ExternalInput").ap()

    p_slots = nc.dram_tensor("pruned_slots", [BL, K, D], F32, kind="ExternalOutput").ap()
    p_masks = nc.dram_tensor("pruned_masks", [BL, N, K], F32, kind="ExternalOutput").ap()
    gates_o = nc.dram_tensor("gates", [BL, K], F32, kind="ExternalOutput").ap()
    util_o = nc.dram_tensor("util", [BL, K], F32, kind="ExternalOutput").ap()

    with tile.TileContext(nc) as tc:
        from contextlib import ExitStack

        with ExitStack() as ctx:
            consts = ctx.enter_context(tc.tile_pool(name="consts", bufs=1))
            mpool = ctx.enter_context(tc.tile_pool(name="mpool", bufs=10))
            srpool = ctx.enter_context(tc.tile_pool(name="srpool", bufs=6))
            gpool = ctx.enter_context(tc.tile_pool(name="gpool", bufs=8))
            mlp = ctx.enter_context(tc.tile_pool(name="mlp", bufs=2))
            slotp = ctx.enter_context(tc.tile_pool(name="slotp", bufs=4))
            psu = ctx.enter_context(tc.tile_pool(name="psu", bufs=2, space="PSUM"))
            pss = ctx.enter_context(tc.tile_pool(name="pss", bufs=2, space="PSUM"))

            # --- constants to SBUF (once) ---
            sb_w1n = consts.tile([16, 16], F32)
            nc.sync.dma_start(sb_w1n[:], w1n)
            sb_b1r = consts.tile([16, 16], F32)
            nc.sync.dma_start(sb_b1r[:], b1r)
            sb_w2r = consts.tile([16, 16], F32)
            nc.sync.dma_start(sb_w2r[:], w2r)
            sb_b2r = consts.tile([16, 1], F32)
            nc.sync.dma_start(sb_b2r[:], b2r)
            sb_sel4 = consts.tile([P, GB, GB], cdt)
            nc.sync.dma_start(sb_sel4[:], sel4)
            sb_brow4 = consts.tile([GB, GB, P], F32)
            nc.sync.dma_start(sb_brow4[:], brow4)
            sb_eye = consts.tile([16, 16], F32)
            nc.sync.dma_start(sb_eye[:], eye16)
            # gates transposed to k-partitions, for slot scaling: [K, BL]
            sb_gt = consts.tile([K, BL], F32)

            b0 = 0
            for g, gb in enumerate(GROUPS):
                pu = psu.tile([gb, FREE], F32, tag="pu")
                m_tiles = []
                for i in range(gb):
                    b = b0 + i
                    mt = mpool.tile([P, FREE], cdt, tag="m")
                    src = masks[b].rearrange("(p ni) k -> p (ni k)", p=P)
                    if bf16:
                        nc.gpsimd.dma_start(mt[:], src)  # SWDGE casts f32->bf16
                    else:
                        nc.sync.dma_start(mt[:], src)
                    m_tiles.append(mt)
                    for j in range(FREE // CHUNK):
                        nc.tensor.matmul(
                            pu[:, j * CHUNK:(j + 1) * CHUNK],
                            lhsT=sb_sel4[:, i, :gb],
                            rhs=mt[:, j * CHUNK:(j + 1) * CHUNK],
                            start=(i == 0),
                            stop=(i == gb - 1),
                        )

                # --- gate MLP for this group's batches (partitions 0..gb-1) ---
                ug = mlp.tile([GB, K], F32, tag="ug", name=f"ug{g}")[:gb]  # raw sums over n
                nc.vector.reduce_sum(
                    ug,
                    pu[:].rearrange("p (ni k) -> p k ni", k=K),
                    axis=mybir.AxisListType.X,
                )
                # h = relu(u_raw * (W1/N) + b1): [gb, 16, K]
                h = mlp.tile([GB, 16, K], F32, tag="h", name=f"h{g}")[:gb]
                nc.vector.tensor_mul(
                    h,
                    ug[:, None, :].broadcast_to([gb, 16, K]),
                    sb_w1n[:gb, :][:, :, None].broadcast_to([gb, 16, K]),
                )
                nc.vector.tensor_add(
                    h, h, sb_b1r[:gb, :][:, :, None].broadcast_to([gb, 16, K])
                )
                h2 = mlp.tile([GB, 16, K], F32, tag="h2", name=f"h2_{g}")[:gb]
                nc.scalar.activation(h2, h, mybir.ActivationFunctionType.Relu)
                # hw = h2 * W2 ; gates_pre = sum_j hw
                hw = mlp.tile([GB, 16, K], F32, tag="hw", name=f"hw{g}")[:gb]
                nc.vector.tensor_mul(
                    hw, h2, sb_w2r[:gb, :][:, :, None].broadcast_to([gb, 16, K])
                )
                gpre = mlp.tile([GB, K], F32, tag="gpre", name=f"gpre{g}")[:gb]
                nc.vector.reduce_sum(
                    gpre,
                    hw.rearrange("p j k -> p k j"),
                    axis=mybir.AxisListType.X,
                )
                ggrp = mlp.tile([GB, K], F32, tag="ggrp", name=f"ggrp{g}")[:gb]
                nc.scalar.activation(
                    ggrp,
                    gpre,
                    mybir.ActivationFunctionType.Sigmoid,
                    bias=sb_b2r[:gb, :],
                    scale=1.0,
                )
                # outputs: gates + utilization rows for this group
                nc.scalar.dma_start(gates_o[b0:b0 + gb, :], ggrp)
                utilg = mlp.tile([GB, K], F32, tag="utilg", name=f"utilg{g}")[:gb]
                nc.scalar.mul(utilg, ug, 1.0 / N)
                nc.scalar.dma_start(util_o[b0:b0 + gb, :], utilg)

                # gates -> [K, gb] slice of sb_gt (for slot scaling)
                pgt = pss.tile([K, GB], F32, tag="pg", name=f"pgt{g}")[:, :gb]
                nc.tensor.transpose(pgt, ggrp, sb_eye[:gb, :gb])
                nc.scalar.copy(sb_gt[:, b0:b0 + gb], pgt)

                # broadcast each batch's gates to all 128 partitions
                greps = []
                for i in range(gb):
                    pg = pss.tile([P, K], F32, tag="pg")
                    nc.tensor.matmul(
                        pg[:], lhsT=sb_brow4[:gb, i, :], rhs=ggrp,
                        start=True, stop=True,
                    )
                    grep = gpool.tile([P, K], cdt, tag="grep")
                    nc.scalar.copy(grep[:], pg[:])
                    greps.append(grep)

                # --- normalize + scale this group's mask tiles ---
                for i in range(gb):
                    b = b0 + i
                    mt = m_tiles[i]
                    m3 = mt[:].rearrange("p (ni k) -> p ni k", k=K)
                    nc.vector.tensor_mul(
                        m3, m3, greps[i][:][:, None, :].broadcast_to([P, NI, K])
                    )
                    s = srpool.tile([P, NI], F32, tag="s")
                    nc.vector.reduce_sum(s[:], m3, axis=mybir.AxisListType.X)
                    r = srpool.tile([P, NI], F32, tag="r")
                    nc.vector.reciprocal_approx_fast(out=r[:], in_=s[:])
                    if bf16:
                        rb = srpool.tile([P, NI], BF16, tag="rb")
                        nc.scalar.copy(rb[:], r[:])
                        rr = rb
                    else:
                        rr = r
                    nc.vector.tensor_mul(
                        m3, m3, rr[:][:, :, None].broadcast_to([P, NI, K])
                    )
                    dst = p_masks[b].rearrange("(p ni) k -> p (ni k)", p=P)
                    if bf16:
                        nc.gpsimd.dma_start(dst, mt[:])  # SWDGE casts bf16->f32
                    else:
                        nc.scalar.dma_start(dst, mt[:])
                b0 += gb

            # --- slots: pruned_slots[b] = slots[b] * gates[b,k] (on ACT) ---
            for b in range(BL):
                st = slotp.tile([K, D], F32, tag="st")
                nc.sync.dma_start(st[:], slots[b])
                st2 = slotp.tile([K, D], F32, tag="st2")
                nc.scalar.mul(st2[:], st[:], sb_gt[:, b:b + 1])
                nc.scalar.dma_start(p_slots[b], st2[:])

    return nc


def make_host_consts(W1, b1, W2, b2, compute_dtype: str = "f32"):
    cnp = np.dtype("bfloat16") if compute_dtype == "bf16" else np.float32
    try:
        import ml_dtypes  # noqa: F401
        if compute_dtype == "bf16":
            cnp = ml_dtypes.bfloat16
    except ImportError:
        pass
    w1n = np.tile((np.asarray(W1, np.float32).reshape(-1) / N), (16, 1)).astype(np.float32)
    b1r = np.tile(np.asarray(b1, np.float32).reshape(-1), (16, 1)).astype(np.float32)
    w2r = np.tile(np.asarray(W2, np.float32).reshape(-1), (16, 1)).astype(np.float32)
    b2r = np.full((16, 1), float(np.asarray(b2).reshape(-1)[0]), np.float32)
    sel4 = np.zeros((P, GB, GB), np.float32)
    for i in range(GB):
        sel4[:, i, i] = 1.0
    brow4 = np.zeros((GB, GB, P), np.float32)
    for i in range(GB):
        brow4[i, i, :] = 1.0
    eye = np.eye(16, dtype=np.float32)
    return {
        "w1n": w1n,
        "b1r": b1r,
        "w2r": w2r,
        "b2r": b2r,
        "sel4": sel4.astype(cnp),
        "brow4": brow4,
        "eye16": eye,
    }


_CACHE = {}


def _get_nc(compute_dtype: str):
    key = compute_dtype
    if key not in _CACHE:
        nc = build_nc(compute_dtype)
        nc.compile()
        _CACHE[key] = nc
    return _CACHE[key]


COMPUTE_DTYPE = "f32"


def kernel(slots, masks, W1, b1, W2, b2, _trace=False, _trace_kwargs=None):
    nc = _get_nc(COMPUTE_DTYPE)
    consts = make_host_consts(W1, b1, W2, b2, COMPUTE_DTYPE)
    masks = np.ascontiguousarray(np.asarray(masks, np.float32))
    slots = np.ascontiguousarray(np.asarray(slots, np.float32))
    in_maps = []
    for c in range(NC):
        m = dict(consts)
        m["masks"] = masks[c * BL:(c + 1) * BL]
        m["slots"] = slots[c * BL:(c + 1) * BL]
        in_maps.append(m)
    kw = {}
    if _trace:
        kw["trace"] = True
        kw.update(_trace_kwargs or {})
    res = run_bass_kernel_spmd(nc, in_maps, list(range(NC)), **kw)
    outs = res.results
    pruned_slots = np.concatenate([outs[c]["pruned_slots"] for c in range(NC)], axis=0)
    pruned_masks = np.concatenate([outs[c]["pruned_masks"] for c in range(NC)], axis=0)
    gates = np.concatenate([outs[c]["gates"] for c in range(NC)], axis=0)
    util = np.concatenate([outs[c]["util"] for c in range(NC)], axis=0)
    kernel.last_results = res
    return (
        pruned_slots.astype(np.float32),
        pruned_masks.astype(np.float32),
        gates.astype(np.float32),
        util.astype(np.float32),
    )


# revision 12
# speedup vs baseline: 1.2713x; 1.1029x over previous
"""AdaptiveSlotPruning Trainium2 kernel (8 NeuronCores, pure data parallel).

Full-input interface: kernel(**inputs) takes the unsharded numpy inputs
(slots [128,12,256], masks [128,16384,12], W1 [1,16], b1 [16], W2 [16,1],
b2 [1]) and returns (pruned_slots, pruned_masks, gates, utilization) as
full-shape float32 arrays, matching reference().

Internally the batch dim (128) is sharded 16-per-core across 8 cores; the
gate-MLP params are replicated. No cross-core communication.
"""

import sys

sys.path.insert(0, "/opt/trn_rl_repo")

import numpy as np

import concourse.bass as bass  # noqa: F401  (re-exported for tests)
import concourse.tile as tile
from concourse import bacc, mybir
from concourse.bass_utils import run_bass_kernel_spmd

F32 = mybir.dt.float32
BF16 = mybir.dt.bfloat16

# Problem constants (hardcoded per spec).
B = 128          # full batch
NC = 8           # cores
BL = B // NC     # 16 batches per core
N = 16384
K = 12
D = 256
P = 128          # partitions
NI = N // P      # 128 inner rows per partition
FREE = NI * K    # 1536 free elements per partition per batch
GB = 4           # max batches per group
GROUPS = [2, 2, 4, 4, 4]   # group sizes (sum = BL); small first groups cut pipeline fill
CHUNK = 512      # matmul moving free-dim max (fp32)
REDUCE_ON_PE = True   # k-rowsum via PE transpose-accumulate instead of DVE reduce
OUT_F32 = True        # final multiply writes f32 -> HWDGE stores (no SWDGE cast)


def build_nc(compute_dtype: str = "f32"):
    """Build the per-core Bass program. Same program on all 8 cores."""
    bf16 = compute_dtype == "bf16"
    cdt = BF16 if bf16 else F32

    nc = bacc.Bacc("TRN2", target_bir_lowering=False, debug=False, num_devices=NC)

    masks = nc.dram_tensor("masks", [BL, N, K], F32, kind="ExternalInput").ap()
    slots = nc.dram_tensor("slots", [BL, K, D], F32, kind="ExternalInput").ap()
    w1n = nc.dram_tensor("w1n", [16, 16], F32, kind="ExternalInput").ap()
    b1r = nc.dram_tensor("b1r", [16, 16], F32, kind="ExternalInput").ap()
    w2r = nc.dram_tensor("w2r", [16, 16], F32, kind="ExternalInput").ap()
    b2r = nc.dram_tensor("b2r", [16, 1], F32, kind="ExternalInput").ap()
    sel4 = nc.dram_tensor("sel4", [P, GB, GB], cdt, kind="ExternalInput").ap()
    brow4 = nc.dram_tensor("brow4", [GB, GB, P], F32, kind="ExternalInput").ap()
    eye16 = nc.dram_tensor("eye16", [16, 16], F32, kind="ExternalInput").ap()
    eye128c = nc.dram_tensor("eye128c", [P, P], cdt, kind="ExternalInput").ap()
    eye128f = nc.dram_tensor("eye128f", [P, P], F32, kind="ExternalInput").ap()

    p_slots = nc.dram_tensor("pruned_slots", [BL, K, D], F32, kind="ExternalOutput").ap()
    p_masks = nc.dram_tensor("pruned_masks", [BL, N, K], F32, kind="ExternalOutput").ap()
    gates_o = nc.dram_tensor("gates", [BL, K], F32, kind="ExternalOutput").ap()
    util_o = nc.dram_tensor("util", [BL, K], F32, kind="ExternalOutput").ap()

    with tile.TileContext(nc) as tc:
        from contextlib import ExitStack

        with ExitStack() as ctx:
            consts = ctx.enter_context(tc.tile_pool(name="consts", bufs=1))
            mpool = ctx.enter_context(tc.tile_pool(name="mpool", bufs=10))
            opool = ctx.enter_context(tc.tile_pool(name="opool", bufs=6))
            srpool = ctx.enter_context(tc.tile_pool(name="srpool", bufs=6))
            gpool = ctx.enter_context(tc.tile_pool(name="gpool", bufs=8))
            mlp = ctx.enter_context(tc.tile_pool(name="mlp", bufs=2))
            slotp = ctx.enter_context(tc.tile_pool(name="slotp", bufs=4))
            psu = ctx.enter_context(tc.tile_pool(name="psu", bufs=1, space="PSUM"))
            pss = ctx.enter_context(tc.tile_pool(name="pss", bufs=2, space="PSUM"))
            psT = ctx.enter_context(tc.tile_pool(name="psT", bufs=1, space="PSUM"))
            psr = ctx.enter_context(tc.tile_pool(name="psr", bufs=2, space="PSUM"))

            # --- constants to SBUF (once) ---
            sb_w1n = consts.tile([16, 16], F32)
            nc.sync.dma_start(sb_w1n[:], w1n)
            sb_b1r = consts.tile([16, 16], F32)
            nc.sync.dma_start(sb_b1r[:], b1r)
            sb_w2r = consts.tile([16, 16], F32)
            nc.sync.dma_start(sb_w2r[:], w2r)
            sb_b2r = consts.tile([16, 1], F32)
            nc.sync.dma_start(sb_b2r[:], b2r)
            sb_sel4 = consts.tile([P, GB, GB], cdt)
            nc.sync.dma_start(sb_sel4[:], sel4)
            sb_brow4 = consts.tile([GB, GB, P], F32)
            nc.sync.dma_start(sb_brow4[:], brow4)
            sb_eye = consts.tile([16, 16], F32)
            nc.sync.dma_start(sb_eye[:], eye16)
            sb_eye128c = consts.tile([P, P], cdt)
            # first SWDGE use pays the Q7 IRAM load (~6us); warm it here
            nc.gpsimd.dma_start(sb_eye128c[:], eye128c)
            sb_eye128f = consts.tile([P, P], F32)
            nc.sync.dma_start(sb_eye128f[:], eye128f)
            # gates transposed to k-partitions, for slot scaling: [K, BL]
            sb_gt = consts.tile([K, BL], F32)

            b0 = 0
            for g, gb in enumerate(GROUPS):
                pu = psu.tile([gb, FREE], F32, tag="pu")
                m_tiles = []
                for i in range(gb):
                    b = b0 + i
                    mt = mpool.tile([P, FREE], cdt, tag="m")
                    src = masks[b].rearrange("(p ni) k -> p (ni k)", p=P)
                    if bf16:
                        nc.gpsimd.dma_start(mt[:], src)  # SWDGE casts f32->bf16
                    else:
                        nc.sync.dma_start(mt[:], src)
                    m_tiles.append(mt)
                    for j in range(FREE // CHUNK):
                        nc.tensor.matmul(
                            pu[:, j * CHUNK:(j + 1) * CHUNK],
                            lhsT=sb_sel4[:, i, :gb],
                            rhs=mt[:, j * CHUNK:(j + 1) * CHUNK],
                            start=(i == 0),
                            stop=(i == gb - 1),
                        )

                # --- gate MLP for this group's batches (partitions 0..gb-1) ---
                ug = mlp.tile([GB, K], F32, tag="ug", name=f"ug{g}")[:gb]  # raw sums over n
                nc.vector.reduce_sum(
                    ug,
                    pu[:].rearrange("p (ni k) -> p k ni", k=K),
                    axis=mybir.AxisListType.X,
                )
                # h = relu(u_raw * (W1/N) + b1): [gb, 16, K]
                h = mlp.tile([GB, 16, K], F32, tag="h", name=f"h{g}")[:gb]
                nc.vector.tensor_mul(
                    h,
                    ug[:, None, :].broadcast_to([gb, 16, K]),
                    sb_w1n[:gb, :][:, :, None].broadcast_to([gb, 16, K]),
                )
                nc.vector.tensor_add(
                    h, h, sb_b1r[:gb, :][:, :, None].broadcast_to([gb, 16, K])
                )
                h2 = mlp.tile([GB, 16, K], F32, tag="h2", name=f"h2_{g}")[:gb]
                nc.scalar.activation(h2, h, mybir.ActivationFunctionType.Relu)
                # hw = h2 * W2 ; gates_pre = sum_j hw
                hw = mlp.tile([GB, 16, K], F32, tag="hw", name=f"hw{g}")[:gb]
                nc.vector.tensor_mul(
                    hw, h2, sb_w2r[:gb, :][:, :, None].broadcast_to([gb, 16, K])
                )
                gpre = mlp.tile([GB, K], F32, tag="gpre", name=f"gpre{g}")[:gb]
                nc.vector.reduce_sum(
                    gpre,
                    hw.rearrange("p j k -> p k j"),
                    axis=mybir.AxisListType.X,
                )
                ggrp = mlp.tile([GB, K], F32, tag="ggrp", name=f"ggrp{g}")[:gb]
                nc.scalar.activation(
                    ggrp,
                    gpre,
                    mybir.ActivationFunctionType.Sigmoid,
                    bias=sb_b2r[:gb, :],
                    scale=1.0,
                )
                # outputs: gates + utilization rows for this group
                nc.scalar.dma_start(gates_o[b0:b0 + gb, :], ggrp)
                utilg = mlp.tile([GB, K], F32, tag="utilg", name=f"utilg{g}")[:gb]
                nc.scalar.mul(utilg, ug, 1.0 / N)
                nc.scalar.dma_start(util_o[b0:b0 + gb, :], utilg)

                # gates -> [K, gb] slice of sb_gt (for slot scaling)
                pgt = pss.tile([K, GB], F32, tag="pg", name=f"pgt{g}")[:, :gb]
                nc.tensor.transpose(pgt, ggrp, sb_eye[:gb, :gb])
                nc.scalar.copy(sb_gt[:, b0:b0 + gb], pgt)

                # broadcast each batch's gates to all 128 partitions
                greps = []
                for i in range(gb):
                    pg = pss.tile([P, K], F32, tag="pg")
                    nc.tensor.matmul(
                        pg[:], lhsT=sb_brow4[:gb, i, :], rhs=ggrp,
                        start=True, stop=True,
                    )
                    grep = gpool.tile([P, K], cdt, tag="grep")
                    nc.scalar.copy(grep[:], pg[:])
                    greps.append(grep)

                # --- normalize + scale this group's mask tiles ---
                for i in range(gb):
                    b = b0 + i
                    mt = m_tiles[i]
                    m3 = mt[:].rearrange("p (ni k) -> p ni k", k=K)
                    nc.vector.tensor_mul(
                        m3, m3, greps[i][:][:, None, :].broadcast_to([P, NI, K])
                    )
                    if REDUCE_ON_PE:
                        # s^T[ni,p] = sum_k t[p,ni,k]: accumulate 12 PE
                        # transposes (lhsT = k-slice of t, rhs = identity)
                        pT = psT.tile([P, NI], F32, tag="pT")
                        tkn = mt[:].rearrange("p (ni k) -> p k ni", k=K)
                        for j in range(K):
                            nc.tensor.matmul(
                                pT[:], lhsT=tkn[:, j, :], rhs=sb_eye128c[:],
                                start=(j == 0), stop=(j == K - 1),
                            )
                        rT = srpool.tile([NI, P], F32, tag="rT")
                        nc.vector.reciprocal_approx_fast(out=rT[:], in_=pT[:])
                        pr = psr.tile([P, NI], F32, tag="pr")
                        nc.tensor.matmul(
                            pr[:], lhsT=rT[:], rhs=sb_eye128f[:],
                            start=True, stop=True,
                        )
                        rr = pr
                    else:
                        s = srpool.tile([P, NI], F32, tag="s")
                        nc.vector.reduce_sum(s[:], m3, axis=mybir.AxisListType.X)
                        r = srpool.tile([P, NI], F32, tag="r")
                        nc.vector.reciprocal_approx_fast(out=r[:], in_=s[:])
                        rr = r
                    if OUT_F32:
                        ot = opool.tile([P, FREE], F32, tag="ot")
                        o3 = ot[:].rearrange("p (ni k) -> p ni k", k=K)
                        nc.vector.tensor_mul(
                            o3, m3, rr[:][:, :, None].broadcast_to([P, NI, K])
                        )
                        dst = p_masks[b].rearrange("(p ni) k -> p (ni k)", p=P)
                        if b % 2 == 0:
                            nc.sync.dma_start(dst, ot[:])
                        else:
                            nc.scalar.dma_start(dst, ot[:])
                    else:
                        if bf16:
                            rb = srpool.tile([P, NI], BF16, tag="rb")
                            nc.scalar.copy(rb[:], rr[:])
                            rr = rb
                        nc.vector.tensor_mul(
                            m3, m3, rr[:][:, :, None].broadcast_to([P, NI, K])
                        )
                        dst = p_masks[b].rearrange("(p ni) k -> p (ni k)", p=P)
                        if bf16:
                            nc.gpsimd.dma_start(dst, mt[:])  # SWDGE cast bf16->f32
                        else:
                            nc.scalar.dma_start(dst, mt[:])
                b0 += gb

            # --- slots: pruned_slots[b] = slots[b] * gates[b,k] (on ACT) ---
            for b in range(BL):
                st = slotp.tile([K, D], F32, tag="st")
                nc.sync.dma_start(st[:], slots[b])
                st2 = slotp.tile([K, D], F32, tag="st2")
                nc.scalar.mul(st2[:], st[:], sb_gt[:, b:b + 1])
                nc.scalar.dma_start(p_slots[b], st2[:])

    return nc


def make_host_consts(W1, b1, W2, b2, compute_dtype: str = "f32"):
    cnp = np.dtype("bfloat16") if compute_dtype == "bf16" else np.float32
    try:
        import ml_dtypes  # noqa: F401
        if compute_dtype == "bf16":
            cnp = ml_dtypes.bfloat16
    except ImportError:
        pass
    w1n = np.tile((np.asarray(W1, np.float32).reshape(-1) / N), (16, 1)).astype(np.float32)
    b1r = np.tile(np.asarray(b1, np.float32).reshape(-1), (16, 1)).astype(np.float32)
    w2r = np.tile(np.asarray(W2, np.float32).reshape(-1), (16, 1)).astype(np.float32)
    b2r = np.full((16, 1), float(np.asarray(b2).reshape(-1)[0]), np.float32)
    sel4 = np.zeros((P, GB, GB), np.float32)
    for i in range(GB):
        sel4[:, i, i] = 1.0
    brow4 = np.zeros((GB, GB, P), np.float32)
    for i in range(GB):
        brow4[i, i, :] = 1.0
    eye = np.eye(16, dtype=np.float32)
    eye128 = np.eye(P, dtype=np.float32)
    return {
        "w1n": w1n,
        "b1r": b1r,
        "w2r": w2r,
        "b2r": b2r,
        "sel4": sel4.astype(cnp),
        "brow4": brow4,
        "eye16": eye,
        "eye128c": eye128.astype(cnp),
        "eye128f": eye128,
    }


_CACHE = {}


def _get_nc(compute_dtype: str):
    key = compute_dtype
    if key not in _CACHE:
        nc = build_nc(compute_dtype)
        nc.compile()
        _CACHE[key] = nc
    return _CACHE[key]


COMPUTE_DTYPE = "f32"


def kernel(slots, masks, W1, b1, W2, b2, _trace=False, _trace_kwargs=None):
    nc = _get_nc(COMPUTE_DTYPE)
    consts = make_host_consts(W1, b1, W2, b2, COMPUTE_DTYPE)
    masks = np.ascontiguousarray(np.asarray(masks, np.float32))
    slots = np.ascontiguousarray(np.asarray(slots, np.float32))
    in_maps = []
    for c in range(NC):
        m = dict(consts)
        m["masks"] = masks[c * BL:(c + 1) * BL]
        m["slots"] = slots[c * BL:(c + 1) * BL]
        in_maps.append(m)
    kw = {}
    if _trace:
        kw["trace"] = True
        kw.update(_trace_kwargs or {})
    res = run_bass_kernel_spmd(nc, in_maps, list(range(NC)), **kw)
    outs = res.results
    pruned_slots = np.concatenate([outs[c]["pruned_slots"] for c in range(NC)], axis=0)
    pruned_masks = np.concatenate([outs[c]["pruned_masks"] for c in range(NC)], axis=0)
    gates = np.concatenate([outs[c]["gates"] for c in range(NC)], axis=0)
    util = np.concatenate([outs[c]["util"] for c in range(NC)], axis=0)
    kernel.last_results = res
    return (
        pruned_slots.astype(np.float32),
        pruned_masks.astype(np.float32),
        gates.astype(np.float32),
        util.astype(np.float32),
    )


# revision 13
# speedup vs baseline: 1.3506x; 1.0624x over previous
"""AdaptiveSlotPruning Trainium2 kernel (8 NeuronCores, pure data parallel).

Full-input interface: kernel(**inputs) takes the unsharded numpy inputs
(slots [128,12,256], masks [128,16384,12], W1 [1,16], b1 [16], W2 [16,1],
b2 [1]) and returns (pruned_slots, pruned_masks, gates, utilization) as
full-shape float32 arrays, matching reference().

Internally the batch dim (128) is sharded 16-per-core across 8 cores; the
gate-MLP params are replicated. No cross-core communication.

Per-core pipeline (memory-bound problem; HBM roofline ~71us/core):
  - masks[b] live in SBUF as [128, 1536] (partition = 128-row chunk of N,
    free = (ni,k)); loads cast f32->bf16 in the DMA (SWDGE).
  - utilization sums via one-hot-column PE matmuls accumulated in PSUM,
    then a tiny gate MLP on <=4 partitions.
  - normalize: t = m*g (DVE 2x), k-rowsum either DVE reduce or PE
    transpose-accumulate, reciprocal_approx_fast, out = t*r via a
    duplicated-r packed AP so the bf16 tensor_tensor stays in 2x mode.
"""

import sys

sys.path.insert(0, "/opt/trn_rl_repo")

import numpy as np

import concourse.bass as bass  # noqa: F401  (re-exported for tests)
import concourse.tile as tile
from concourse import bacc, mybir
from concourse.bass_utils import run_bass_kernel_spmd

F32 = mybir.dt.float32
BF16 = mybir.dt.bfloat16

# Problem constants (hardcoded per spec).
B = 128          # full batch
NC = 8           # cores
BL = B // NC     # 16 batches per core
N = 16384
K = 12
D = 256
P = 128          # partitions
NI = N // P      # 128 inner rows per partition
FREE = NI * K    # 1536 free elements per partition per batch
GB = 4           # max batches per group
GROUPS = [1, 1, 2, 4, 4, 4]  # group sizes (sum = BL); small first groups cut fill
CHUNK = 512      # matmul moving free-dim max
PE_RED_PERIOD = 3  # batches with b % period == 2 do the k-rowsum on PE


def _load_units(gb):
    if gb <= 2:
        return [gb]
    return [2] * (gb // 2)


def build_nc(compute_dtype: str = "bf16"):
    """Build the per-core Bass program. Same program on all 8 cores."""
    bf16 = compute_dtype == "bf16"
    cdt = BF16 if bf16 else F32

    nc = bacc.Bacc("TRN2", target_bir_lowering=False, debug=False, num_devices=NC)

    masks = nc.dram_tensor("masks", [BL, N, K], F32, kind="ExternalInput").ap()
    slots = nc.dram_tensor("slots", [BL, K, D], F32, kind="ExternalInput").ap()
    cpackf = nc.dram_tensor("cpackf", [16, 577], F32, kind="ExternalInput").ap()
    cpackb = nc.dram_tensor("cpackb", [P, 144], cdt, kind="ExternalInput").ap()
    eye128f = nc.dram_tensor("eye128f", [P, P], F32, kind="ExternalInput").ap()

    p_slots = nc.dram_tensor("pruned_slots", [BL, K, D], F32, kind="ExternalOutput").ap()
    p_masks = nc.dram_tensor("pruned_masks", [BL, N, K], F32, kind="ExternalOutput").ap()
    gates_o = nc.dram_tensor("gates", [BL, K], F32, kind="ExternalOutput").ap()
    util_o = nc.dram_tensor("util", [BL, K], F32, kind="ExternalOutput").ap()

    with tile.TileContext(nc) as tc:
        from contextlib import ExitStack

        with ExitStack() as ctx:
            consts = ctx.enter_context(tc.tile_pool(name="consts", bufs=1))
            mpool = ctx.enter_context(tc.tile_pool(name="mpool", bufs=6))
            srpool = ctx.enter_context(tc.tile_pool(name="srpool", bufs=6))
            gpool = ctx.enter_context(tc.tile_pool(name="gpool", bufs=8))
            mlp = ctx.enter_context(tc.tile_pool(name="mlp", bufs=2))
            slotp = ctx.enter_context(tc.tile_pool(name="slotp", bufs=6))
            psu = ctx.enter_context(tc.tile_pool(name="psu", bufs=1, space="PSUM"))
            pss = ctx.enter_context(tc.tile_pool(name="pss", bufs=2, space="PSUM"))
            psT = ctx.enter_context(tc.tile_pool(name="psT", bufs=1, space="PSUM"))
            psr = ctx.enter_context(tc.tile_pool(name="psr", bufs=2, space="PSUM"))

            # --- constants to SBUF (3 packed DMAs) ---
            sb_cf = consts.tile([16, 577], F32)
            nc.scalar.dma_start(sb_cf[:], cpackf)
            sb_cb = consts.tile([P, 144], cdt)
            nc.gpsimd.dma_start(sb_cb[:], cpackb)  # also warms the SWDGE path
            sb_eye128f = consts.tile([P, P], F32)
            nc.sync.dma_start(sb_eye128f[:], eye128f)

            sb_w1n = sb_cf[:, 0:16]
            sb_b1r = sb_cf[:, 16:32]
            sb_w2r = sb_cf[:, 32:48]
            sb_b2r = sb_cf[:, 48:49]
            sb_eye = sb_cf[:, 49:65]
            sb_brow4 = sb_cf[:4, 65:577].rearrange("p (i m) -> p i m", i=GB)
            sb_sel4 = sb_cb[:, 0:16].rearrange("p (i j) -> p i j", i=GB)
            sb_eye128c = sb_cb[:, 16:144]
            # gates transposed to k-partitions, for slot scaling: [K, BL]
            sb_gt = consts.tile([K, BL], F32)

            b0 = 0
            for g, gb in enumerate(GROUPS):
                pu = psu.tile([gb, FREE], F32, tag="pu", name=f"pu{g}")
                m3s = []  # per-batch [P, NI, K] views
                mts = []  # (tile, first_batch, nb) for stores
                i = 0
                for u in _load_units(gb):
                    b = b0 + i
                    mt = mpool.tile([P, u * FREE], cdt, tag="m", name=f"mt{b}")
                    if u == 1:
                        src = masks[b].rearrange("(p ni) k -> p (ni k)", p=P)
                    else:
                        src = masks[b:b + u].rearrange(
                            "b (p ni) k -> p b (ni k)", p=P
                        )
                    if bf16:
                        nc.gpsimd.dma_start(mt[:], src)  # SWDGE casts f32->bf16
                    else:
                        nc.sync.dma_start(mt[:], src)
                    mts.append((mt, b, u))
                    for v in range(u):
                        m3s.append(
                            mt[:, v * FREE:(v + 1) * FREE].rearrange(
                                "p (ni k) -> p ni k", k=K
                            )
                        )
                        for j in range(FREE // CHUNK):
                            nc.tensor.matmul(
                                pu[:, j * CHUNK:(j + 1) * CHUNK],
                                lhsT=sb_sel4[:, i + v, :gb],
                                rhs=mt[:, (v * FREE + j * CHUNK):(v * FREE + (j + 1) * CHUNK)],
                                start=(i + v == 0),
                                stop=(i + v == gb - 1),
                            )
                    i += u

                # --- gate MLP for this group's batches (partitions 0..gb-1) ---
                ug = mlp.tile([GB, K], F32, tag="ug", name=f"ug{g}")[:gb]
                nc.vector.reduce_sum(
                    ug,
                    pu[:].rearrange("p (ni k) -> p k ni", k=K),
                    axis=mybir.AxisListType.X,
                )
                h = mlp.tile([GB, 16, K], F32, tag="h", name=f"h{g}")[:gb]
                nc.vector.tensor_mul(
                    h,
                    ug[:, None, :].broadcast_to([gb, 16, K]),
                    sb_w1n[:gb, :][:, :, None].broadcast_to([gb, 16, K]),
                )
                nc.vector.tensor_add(
                    h, h, sb_b1r[:gb, :][:, :, None].broadcast_to([gb, 16, K])
                )
                h2 = mlp.tile([GB, 16, K], F32, tag="h2", name=f"h2_{g}")[:gb]
                nc.scalar.activation(h2, h, mybir.ActivationFunctionType.Relu)
                hw = mlp.tile([GB, 16, K], F32, tag="hw", name=f"hw{g}")[:gb]
                nc.vector.tensor_mul(
                    hw, h2, sb_w2r[:gb, :][:, :, None].broadcast_to([gb, 16, K])
                )
                gpre = mlp.tile([GB, K], F32, tag="gpre", name=f"gpre{g}")[:gb]
                nc.vector.reduce_sum(
                    gpre,
                    hw.rearrange("p j k -> p k j"),
                    axis=mybir.AxisListType.X,
                )
                ggrp = mlp.tile([GB, K], F32, tag="ggrp", name=f"ggrp{g}")[:gb]
                nc.scalar.activation(
                    ggrp,
                    gpre,
                    mybir.ActivationFunctionType.Sigmoid,
                    bias=sb_b2r[:gb, :],
                    scale=1.0,
                )
                nc.scalar.dma_start(gates_o[b0:b0 + gb, :], ggrp)
                utilg = mlp.tile([GB, K], F32, tag="utilg", name=f"utilg{g}")[:gb]
                nc.scalar.mul(utilg, ug, 1.0 / N)
                nc.scalar.dma_start(util_o[b0:b0 + gb, :], utilg)

                # gates -> [K, gb] slice of sb_gt (for slot scaling)
                pgt = pss.tile([K, GB], F32, tag="pg", name=f"pgt{g}")[:, :gb]
                nc.tensor.transpose(pgt, ggrp, sb_eye[:gb, :gb])
                nc.scalar.copy(sb_gt[:, b0:b0 + gb], pgt)

                # broadcast each batch's gates to all 128 partitions
                greps = []
                for i in range(gb):
                    pg = pss.tile([P, K], F32, tag="pg", name=f"pg{g}_{i}")
                    nc.tensor.matmul(
                        pg[:], lhsT=sb_brow4[:gb, i, :], rhs=ggrp,
                        start=True, stop=True,
                    )
                    grep = gpool.tile([P, K], cdt, tag="grep", name=f"grep{g}_{i}")
                    nc.scalar.copy(grep[:], pg[:])
                    greps.append(grep)

                # --- normalize + scale this group's mask tiles ---
                for i in range(gb):
                    b = b0 + i
                    m3 = m3s[i]
                    nc.vector.tensor_mul(
                        m3, m3, greps[i][:][:, None, :].broadcast_to([P, NI, K])
                    )
                    # rb2: r duplicated pairwise [P, 2*NI] in bf16, so the
                    # final tensor_tensor keeps innermost step 1 (2x mode)
                    rb2 = srpool.tile([P, 2 * NI], cdt, tag="rb2", name=f"rb2_{b}")
                    rb2d = rb2[:].rearrange("p (ni d) -> p ni d", d=2)
                    if b % PE_RED_PERIOD == 2:
                        # k-rowsum on PE: accumulate 12 transposes, then
                        # transpose the reciprocal back.
                        pT = psT.tile([P, NI], F32, tag="pT", name=f"pT{b}")
                        tkn = m3.rearrange("p ni k -> p k ni")
                        for j in range(K):
                            nc.tensor.matmul(
                                pT[:], lhsT=tkn[:, j, :], rhs=sb_eye128c[:],
                                start=(j == 0), stop=(j == K - 1),
                            )
                        rT = srpool.tile([NI, P], F32, tag="rT", name=f"rT{b}")
                        nc.vector.reciprocal_approx_fast(out=rT[:], in_=pT[:])
                        pr = psr.tile([P, NI], F32, tag="pr", name=f"pr{b}")
                        nc.tensor.matmul(
                            pr[:], lhsT=rT[:], rhs=sb_eye128f[:],
                            start=True, stop=True,
                        )
                        nc.scalar.copy(
                            rb2d,
                            pr[:][:, :, None].broadcast_to([P, NI, 2]),
                        )
                    else:
                        s = srpool.tile([P, NI], F32, tag="s", name=f"s{b}")
                        nc.vector.reduce_sum(s[:], m3, axis=mybir.AxisListType.X)
                        r = srpool.tile([P, NI], F32, tag="r", name=f"r{b}")
                        nc.vector.reciprocal_approx_fast(out=r[:], in_=s[:])
                        nc.scalar.copy(
                            rb2d,
                            r[:][:, :, None].broadcast_to([P, NI, 2]),
                        )
                    # out = t * r, all operands innermost step 1 bf16
                    m4 = m3.rearrange("p ni (k6 d) -> p ni k6 d", d=2)
                    rb4 = rb2[:].rearrange("p (ni d) -> p ni d", d=2)[
                        :, :, None, :
                    ].broadcast_to([P, NI, K // 2, 2])
                    nc.vector.tensor_mul(m4, m4, rb4)

                # stores (SWDGE casts bf16->f32), paired like the loads
                for mt, b, u in mts:
                    if u == 1:
                        dst = p_masks[b].rearrange("(p ni) k -> p (ni k)", p=P)
                    else:
                        dst = p_masks[b:b + u].rearrange(
                            "b (p ni) k -> p b (ni k)", p=P
                        )
                    if bf16:
                        nc.gpsimd.dma_start(dst, mt[:])
                    else:
                        nc.scalar.dma_start(dst, mt[:])

                # slots for this group (gates for these batches are final)
                for i in range(gb):
                    b = b0 + i
                    st = slotp.tile([K, D], F32, tag="st", name=f"st{b}")
                    nc.sync.dma_start(st[:], slots[b])
                    st2 = slotp.tile([K, D], F32, tag="st2", name=f"st2_{b}")
                    nc.scalar.mul(st2[:], st[:], sb_gt[:, b:b + 1])
                    nc.scalar.dma_start(p_slots[b], st2[:])
                b0 += gb

    return nc


def make_host_consts(W1, b1, W2, b2, compute_dtype: str = "bf16"):
    cnp = np.float32
    if compute_dtype == "bf16":
        import ml_dtypes
        cnp = ml_dtypes.bfloat16
    w1n = np.tile((np.asarray(W1, np.float32).reshape(-1) / N), (16, 1))
    b1r = np.tile(np.asarray(b1, np.float32).reshape(-1), (16, 1))
    w2r = np.tile(np.asarray(W2, np.float32).reshape(-1), (16, 1))
    b2r = np.full((16, 1), float(np.asarray(b2).reshape(-1)[0]), np.float32)
    eye = np.eye(16, dtype=np.float32)
    brow4 = np.zeros((GB, GB, P), np.float32)
    for i in range(GB):
        brow4[i, i, :] = 1.0
    cpackf = np.zeros((16, 577), np.float32)
    cpackf[:, 0:16] = w1n
    cpackf[:, 16:32] = b1r
    cpackf[:, 32:48] = w2r
    cpackf[:, 48:49] = b2r
    cpackf[:, 49:65] = eye
    cpackf[:4, 65:577] = brow4.reshape(GB, GB * P)
    sel4 = np.zeros((P, GB, GB), np.float32)
    for i in range(GB):
        sel4[:, i, i] = 1.0
    cpackb = np.zeros((P, 144), np.float32)
    cpackb[:, 0:16] = sel4.reshape(P, 16)
    cpackb[:, 16:144] = np.eye(P, dtype=np.float32)
    return {
        "cpackf": cpackf,
        "cpackb": cpackb.astype(cnp),
        "eye128f": np.eye(P, dtype=np.float32),
    }


_CACHE = {}


def _get_nc(compute_dtype: str):
    key = compute_dtype
    if key not in _CACHE:
        nc = build_nc(compute_dtype)
        nc.compile()
        _CACHE[key] = nc
    return _CACHE[key]


COMPUTE_DTYPE = "bf16"


def kernel(slots, masks, W1, b1, W2, b2, _trace=False, _trace_kwargs=None):
    nc = _get_nc(COMPUTE_DTYPE)
    consts = make_host_consts(W1, b1, W2, b2, COMPUTE_DTYPE)
    masks = np.ascontiguousarray(np.asarray(masks, np.float32))
    slots = np.ascontiguousarray(np.asarray(slots, np.float32))
    in_maps = []
    for c in range(NC):
        m = dict(consts)
        m["masks"] = masks[c * BL:(c + 1) * BL]
        m["slots"] = slots[c * BL:(c + 1) * BL]
        in_maps.append(m)
    kw = {}
    if _trace:
        kw["trace"] = True
        kw.update(_trace_kwargs or {})
    res = run_bass_kernel_spmd(nc, in_maps, list(range(NC)), **kw)
    outs = res.results
    pruned_slots = np.concatenate([outs[c]["pruned_slots"] for c in range(NC)], axis=0)
    pruned_masks = np.concatenate([outs[c]["pruned_masks"] for c in range(NC)], axis=0)
    gates = np.concatenate([outs[c]["gates"] for c in range(NC)], axis=0)
    util = np.concatenate([outs[c]["util"] for c in range(NC)], axis=0)
    kernel.last_results = res
    return (
        pruned_slots.astype(np.float32),
        pruned_masks.astype(np.float32),
        gates.astype(np.float32),
        util.astype(np.float32),
    )


# revision 14
# speedup vs baseline: 1.3585x; 1.0059x over previous
"""AdaptiveSlotPruning Trainium2 kernel (8 NeuronCores, pure data parallel).

Full-input interface: kernel(**inputs) takes the unsharded numpy inputs
(slots [128,12,256], masks [128,16384,12], W1 [1,16], b1 [16], W2 [16,1],
b2 [1]) and returns (pruned_slots, pruned_masks, gates, utilization) as
full-shape float32 arrays, matching reference().

Internally the batch dim (128) is sharded 16-per-core across 8 cores; the
gate-MLP params are replicated. No cross-core communication.

Per-core pipeline (memory-bound problem; HBM roofline ~71us/core):
  - masks[b] live in SBUF as [128, 1536] (partition = 128-row chunk of N,
    free = (ni,k)); loads cast f32->bf16 in the DMA (SWDGE).
  - utilization sums via one-hot-column PE matmuls accumulated in PSUM,
    then a tiny gate MLP on <=4 partitions.
  - normalize: t = m*g (DVE 2x), k-rowsum either DVE reduce or PE
    transpose-accumulate, reciprocal_approx_fast, out = t*r via a
    duplicated-r packed AP so the bf16 tensor_tensor stays in 2x mode.
"""

import sys

sys.path.insert(0, "/opt/trn_rl_repo")

import numpy as np

import concourse.bass as bass  # noqa: F401  (re-exported for tests)
import concourse.tile as tile
from concourse import bacc, mybir
from concourse.bass_utils import run_bass_kernel_spmd

F32 = mybir.dt.float32
BF16 = mybir.dt.bfloat16

# Problem constants (hardcoded per spec).
B = 128          # full batch
NC = 8           # cores
BL = B // NC     # 16 batches per core
N = 16384
K = 12
D = 256
P = 128          # partitions
NI = N // P      # 128 inner rows per partition
FREE = NI * K    # 1536 free elements per partition per batch
GB = 4           # max batches per group
GROUPS = [1, 1, 2, 4, 4, 2, 2]  # group sizes (sum = BL); small first groups cut fill
CHUNK = 512      # matmul moving free-dim max
PE_RED_PERIOD = 3  # batches with b % period == 2 do the k-rowsum on PE


def _load_units(gb):
    if gb <= 2:
        return [gb]
    return [2] * (gb // 2)


def build_nc(compute_dtype: str = "bf16"):
    """Build the per-core Bass program. Same program on all 8 cores."""
    bf16 = compute_dtype == "bf16"
    cdt = BF16 if bf16 else F32

    nc = bacc.Bacc("TRN2", target_bir_lowering=False, debug=False, num_devices=NC)

    masks = nc.dram_tensor("masks", [BL, N, K], F32, kind="ExternalInput").ap()
    slots = nc.dram_tensor("slots", [BL, K, D], F32, kind="ExternalInput").ap()
    cpackf = nc.dram_tensor("cpackf", [16, 577], F32, kind="ExternalInput").ap()
    cpackb = nc.dram_tensor("cpackb", [P, 144], cdt, kind="ExternalInput").ap()
    eye128f = nc.dram_tensor("eye128f", [P, P], F32, kind="ExternalInput").ap()

    p_slots = nc.dram_tensor("pruned_slots", [BL, K, D], F32, kind="ExternalOutput").ap()
    p_masks = nc.dram_tensor("pruned_masks", [BL, N, K], F32, kind="ExternalOutput").ap()
    gates_o = nc.dram_tensor("gates", [BL, K], F32, kind="ExternalOutput").ap()
    util_o = nc.dram_tensor("util", [BL, K], F32, kind="ExternalOutput").ap()

    with tile.TileContext(nc) as tc:
        from contextlib import ExitStack

        with ExitStack() as ctx:
            consts = ctx.enter_context(tc.tile_pool(name="consts", bufs=1))
            mpool = ctx.enter_context(tc.tile_pool(name="mpool", bufs=10))
            srpool = ctx.enter_context(tc.tile_pool(name="srpool", bufs=6))
            gpool = ctx.enter_context(tc.tile_pool(name="gpool", bufs=8))
            mlp = ctx.enter_context(tc.tile_pool(name="mlp", bufs=2))
            slotp = ctx.enter_context(tc.tile_pool(name="slotp", bufs=6))
            psu = ctx.enter_context(tc.tile_pool(name="psu", bufs=2, space="PSUM"))
            pss = ctx.enter_context(tc.tile_pool(name="pss", bufs=2, space="PSUM"))

            # --- constants to SBUF (3 packed DMAs) ---
            sb_cf = consts.tile([16, 577], F32)
            nc.scalar.dma_start(sb_cf[:], cpackf)
            sb_cb = consts.tile([P, 144], cdt)
            nc.gpsimd.dma_start(sb_cb[:], cpackb)  # also warms the SWDGE path
            sb_eye128f = consts.tile([P, P], F32)
            nc.sync.dma_start(sb_eye128f[:], eye128f)

            sb_w1n = sb_cf[:, 0:16]
            sb_b1r = sb_cf[:, 16:32]
            sb_w2r = sb_cf[:, 32:48]
            sb_b2r = sb_cf[:, 48:49]
            sb_eye = sb_cf[:, 49:65]
            sb_brow4 = sb_cf[:4, 65:577].rearrange("p (i m) -> p i m", i=GB)
            sb_sel4 = sb_cb[:, 0:16].rearrange("p (i j) -> p i j", i=GB)
            sb_eye128c = sb_cb[:, 16:144]
            # gates transposed to k-partitions, for slot scaling: [K, BL]
            sb_gt = consts.tile([K, BL], F32)

            b0 = 0
            for g, gb in enumerate(GROUPS):
                pu = psu.tile([gb, FREE], F32, tag="pu", name=f"pu{g}")
                m3s = []  # per-batch [P, NI, K] views
                mts = []  # (tile, first_batch, nb) for stores
                i = 0
                for u in _load_units(gb):
                    b = b0 + i
                    mt = mpool.tile([P, u * FREE], cdt, tag="m", name=f"mt{b}")
                    dma = nc.gpsimd.dma_start if bf16 else nc.sync.dma_start
                    if u == 1:
                        src = masks[b].rearrange("(p ni) k -> p (ni k)", p=P)
                        if gb == 1:
                            # chunked load: lets util matmuls start early
                            for j in range(FREE // CHUNK):
                                dma(mt[:, j * CHUNK:(j + 1) * CHUNK],
                                    src[:, j * CHUNK:(j + 1) * CHUNK])
                        else:
                            dma(mt[:], src)
                    else:
                        src = masks[b:b + u].rearrange(
                            "b (p ni) k -> p b (ni k)", p=P
                        )
                        dma(mt[:], src)
                    mts.append((mt, b, u))
                    for v in range(u):
                        m3s.append(
                            mt[:, v * FREE:(v + 1) * FREE].rearrange(
                                "p (ni k) -> p ni k", k=K
                            )
                        )
                        for j in range(FREE // CHUNK):
                            nc.tensor.matmul(
                                pu[:, j * CHUNK:(j + 1) * CHUNK],
                                lhsT=sb_sel4[:, i + v, :gb],
                                rhs=mt[:, (v * FREE + j * CHUNK):(v * FREE + (j + 1) * CHUNK)],
                                start=(i + v == 0),
                                stop=(i + v == gb - 1),
                            )
                    i += u

                # --- gate MLP for this group's batches (partitions 0..gb-1) ---
                ug = mlp.tile([GB, K], F32, tag="ug", name=f"ug{g}")[:gb]
                nc.vector.reduce_sum(
                    ug,
                    pu[:].rearrange("p (ni k) -> p k ni", k=K),
                    axis=mybir.AxisListType.X,
                )
                h = mlp.tile([GB, 16, K], F32, tag="h", name=f"h{g}")[:gb]
                nc.vector.tensor_mul(
                    h,
                    ug[:, None, :].broadcast_to([gb, 16, K]),
                    sb_w1n[:gb, :][:, :, None].broadcast_to([gb, 16, K]),
                )
                nc.vector.tensor_add(
                    h, h, sb_b1r[:gb, :][:, :, None].broadcast_to([gb, 16, K])
                )
                h2 = mlp.tile([GB, 16, K], F32, tag="h2", name=f"h2_{g}")[:gb]
                nc.scalar.activation(h2, h, mybir.ActivationFunctionType.Relu)
                hw = mlp.tile([GB, 16, K], F32, tag="hw", name=f"hw{g}")[:gb]
                nc.vector.tensor_mul(
                    hw, h2, sb_w2r[:gb, :][:, :, None].broadcast_to([gb, 16, K])
                )
                gpre = mlp.tile([GB, K], F32, tag="gpre", name=f"gpre{g}")[:gb]
                nc.vector.reduce_sum(
                    gpre,
                    hw.rearrange("p j k -> p k j"),
                    axis=mybir.AxisListType.X,
                )
                ggrp = mlp.tile([GB, K], F32, tag="ggrp", name=f"ggrp{g}")[:gb]
                nc.scalar.activation(
                    ggrp,
                    gpre,
                    mybir.ActivationFunctionType.Sigmoid,
                    bias=sb_b2r[:gb, :],
                    scale=1.0,
                )
                nc.scalar.dma_start(gates_o[b0:b0 + gb, :], ggrp)
                utilg = mlp.tile([GB, K], F32, tag="utilg", name=f"utilg{g}")[:gb]
                nc.scalar.mul(utilg, ug, 1.0 / N)
                nc.scalar.dma_start(util_o[b0:b0 + gb, :], utilg)

                # gates -> [K, gb] slice of sb_gt (for slot scaling)
                pgt = pss.tile([K, GB], F32, tag="pg", name=f"pgt{g}")[:, :gb]
                nc.tensor.transpose(pgt, ggrp, sb_eye[:gb, :gb])
                nc.scalar.copy(sb_gt[:, b0:b0 + gb], pgt)

                # broadcast each batch's gates to all 128 partitions
                greps = []
                for i in range(gb):
                    pg = pss.tile([P, K], F32, tag="pg", name=f"pg{g}_{i}")
                    nc.tensor.matmul(
                        pg[:], lhsT=sb_brow4[:gb, i, :], rhs=ggrp,
                        start=True, stop=True,
                    )
                    grep = gpool.tile([P, K], cdt, tag="grep", name=f"grep{g}_{i}")
                    nc.scalar.copy(grep[:], pg[:])
                    greps.append(grep)

                # --- normalize + scale this group's mask tiles ---
                for i in range(gb):
                    b = b0 + i
                    m3 = m3s[i]
                    nc.vector.tensor_mul(
                        m3, m3, greps[i][:][:, None, :].broadcast_to([P, NI, K])
                    )
                    # rb2: r duplicated pairwise [P, 2*NI] in bf16, so the
                    # final tensor_tensor keeps innermost step 1 (2x mode)
                    rb2 = srpool.tile([P, 2 * NI], cdt, tag="rb2", name=f"rb2_{b}")
                    rb2d = rb2[:].rearrange("p (ni d) -> p ni d", d=2)
                    if b % PE_RED_PERIOD == 2:
                        # k-rowsum on PE: accumulate 12 transposes, then
                        # transpose the reciprocal back.
                        pT = pss.tile([P, NI], F32, tag="pg", name=f"pT{b}")
                        tkn = m3.rearrange("p ni k -> p k ni")
                        for j in range(K):
                            nc.tensor.matmul(
                                pT[:], lhsT=tkn[:, j, :], rhs=sb_eye128c[:],
                                start=(j == 0), stop=(j == K - 1),
                            )
                        rT = srpool.tile([NI, P], F32, tag="rT", name=f"rT{b}")
                        nc.vector.reciprocal_approx_fast(out=rT[:], in_=pT[:])
                        pr = pss.tile([P, NI], F32, tag="pg", name=f"pr{b}")
                        nc.tensor.matmul(
                            pr[:], lhsT=rT[:], rhs=sb_eye128f[:],
                            start=True, stop=True,
                        )
                        nc.scalar.copy(
                            rb2d,
                            pr[:][:, :, None].broadcast_to([P, NI, 2]),
                        )
                    else:
                        s = srpool.tile([P, NI], F32, tag="s", name=f"s{b}")
                        nc.vector.reduce_sum(s[:], m3, axis=mybir.AxisListType.X)
                        r = srpool.tile([P, NI], F32, tag="r", name=f"r{b}")
                        nc.vector.reciprocal_approx_fast(out=r[:], in_=s[:])
                        nc.scalar.copy(
                            rb2d,
                            r[:][:, :, None].broadcast_to([P, NI, 2]),
                        )
                    # out = t * r, all operands innermost step 1 bf16
                    m4 = m3.rearrange("p ni (k6 d) -> p ni k6 d", d=2)
                    rb4 = rb2[:].rearrange("p (ni d) -> p ni d", d=2)[
                        :, :, None, :
                    ].broadcast_to([P, NI, K // 2, 2])
                    nc.vector.tensor_mul(m4, m4, rb4)

                # stores (SWDGE casts bf16->f32), one per batch
                for mt, b, u in mts:
                    for v in range(u):
                        dst = p_masks[b + v].rearrange("(p ni) k -> p (ni k)", p=P)
                        sdma = nc.gpsimd.dma_start if bf16 else nc.scalar.dma_start
                        sdma(dst, mt[:, v * FREE:(v + 1) * FREE])

                # slots for this group (gates for these batches are final)
                for i in range(gb):
                    b = b0 + i
                    st = slotp.tile([K, D], F32, tag="st", name=f"st{b}")
                    nc.sync.dma_start(st[:], slots[b])
                    st2 = slotp.tile([K, D], F32, tag="st2", name=f"st2_{b}")
                    nc.scalar.mul(st2[:], st[:], sb_gt[:, b:b + 1])
                    nc.scalar.dma_start(p_slots[b], st2[:])
                b0 += gb

    return nc


def make_host_consts(W1, b1, W2, b2, compute_dtype: str = "bf16"):
    cnp = np.float32
    if compute_dtype == "bf16":
        import ml_dtypes
        cnp = ml_dtypes.bfloat16
    w1n = np.tile((np.asarray(W1, np.float32).reshape(-1) / N), (16, 1))
    b1r = np.tile(np.asarray(b1, np.float32).reshape(-1), (16, 1))
    w2r = np.tile(np.asarray(W2, np.float32).reshape(-1), (16, 1))
    b2r = np.full((16, 1), float(np.asarray(b2).reshape(-1)[0]), np.float32)
    eye = np.eye(16, dtype=np.float32)
    brow4 = np.zeros((GB, GB, P), np.float32)
    for i in range(GB):
        brow4[i, i, :] = 1.0
    cpackf = np.zeros((16, 577), np.float32)
    cpackf[:, 0:16] = w1n
    cpackf[:, 16:32] = b1r
    cpackf[:, 32:48] = w2r
    cpackf[:, 48:49] = b2r
    cpackf[:, 49:65] = eye
    cpackf[:4, 65:577] = brow4.reshape(GB, GB * P)
    sel4 = np.zeros((P, GB, GB), np.float32)
    for i in range(GB):
        sel4[:, i, i] = 1.0
    cpackb = np.zeros((P, 144), np.float32)
    cpackb[:, 0:16] = sel4.reshape(P, 16)
    cpackb[:, 16:144] = np.eye(P, dtype=np.float32)
    return {
        "cpackf": cpackf,
        "cpackb": cpackb.astype(cnp),
        "eye128f": np.eye(P, dtype=np.float32),
    }


_CACHE = {}


def _get_nc(compute_dtype: str):
    key = compute_dtype
    if key not in _CACHE:
        nc = build_nc(compute_dtype)
        nc.compile()
        _CACHE[key] = nc
    return _CACHE[key]


COMPUTE_DTYPE = "bf16"


def kernel(slots, masks, W1, b1, W2, b2, _trace=False, _trace_kwargs=None):
    nc = _get_nc(COMPUTE_DTYPE)
    consts = make_host_consts(W1, b1, W2, b2, COMPUTE_DTYPE)
    masks = np.ascontiguousarray(np.asarray(masks, np.float32))
    slots = np.ascontiguousarray(np.asarray(slots, np.float32))
    in_maps = []
    for c in range(NC):
        m = dict(consts)
        m["masks"] = masks[c * BL:(c + 1) * BL]
        m["slots"] = slots[c * BL:(c + 1) * BL]
        in_maps.append(m)
    kw = {}
    if _trace:
        kw["trace"] = True
        kw.update(_trace_kwargs or {})
    res = run_bass_kernel_spmd(nc, in_maps, list(range(NC)), **kw)
    outs = res.results
    pruned_slots = np.concatenate([outs[c]["pruned_slots"] for c in range(NC)], axis=0)
    pruned_masks = np.concatenate([outs[c]["pruned_masks"] for c in range(NC)], axis=0)
    gates = np.concatenate([outs[c]["gates"] for c in range(NC)], axis=0)
    util = np.concatenate([outs[c]["util"] for c in range(NC)], axis=0)
    kernel.last_results = res
    return (
        pruned_slots.astype(np.float32),
        pruned_masks.astype(np.float32),
        gates.astype(np.float32),
        util.astype(np.float32),
    )
